# revision 21
# baseline (speedup 1.0000x reference)
"""DiffPool GNN forward on 8 Trainium2 NeuronCores.

Data-parallel over the batch dim (B=16 -> 2 batches per core). Each core
receives its two batches' dense node features (bf16, transposed) and dense
transposed adjacency (fp8e4, 0/1 exact), runs the DiffPool pipeline locally,
and emits its two [2] outputs.

Key structure vs the naive version:
  - adjacency ships as fp8 (half the HBM bytes) and is consumed directly as
    the stationary matmul operand against bf16 softmax scores (mixed-dtype
    matmul, numerically exact for 0/1 weights).
  - the t = adj @ s GEMM is v-outer (streams slabs as they arrive from HBM)
    with 16 PSUM accumulators dual-packed into 4 banks x 2 passes, and skips
    the structurally-zero blocks (g2 rows never see g1 columns): 200 matmuls
    instead of 256, slabs 0..7 shipped at 1152 instead of 2048 columns.
  - level-2 GNNs are algebraically restructured to be transpose-free (every
    operand is produced directly in the layout the next matmul needs).
  - the two batches' level-2 chains are interleaved to hide latency.
"""

import numpy as np
import ml_dtypes

import concourse.bass as bass
import concourse.mybir as mybir
from concourse import tile
from concourse.bass_utils import run_bass_kernel_spmd

# ---------------------------------------------------------------------------
# Problem constants (hardcoded per spec)
# ---------------------------------------------------------------------------
B = 16
NCORES = 8
BPC = B // NCORES          # batches per core
MN = 2048                  # MAX_NODES
IN_DIM = 128
HID = 64
OUT = 2
K1 = 205
K2 = 21
N1P = 1100                 # g1 nodes per batch
N2P = 900                  # g2 nodes per batch
W1COLS = 1152              # trimmed slab width for dst-chunks 0..7
PROJ_N = 272               # 205 scores | 3 pad | 64 emb  (relu offset 8B-aligned)
HOFF = 208                 # emb column offset inside proj psum

F32 = mybir.dt.float32
BF16 = mybir.dt.bfloat16
FP8 = mybir.dt.float8e4

_M2 = ((0, 128), (128, K1 - 128))      # row tiling of a 205-row matrix

L2_W_SHAPES = [
    ("Wp1", [HID, K2]), ("Up1", [HID, K2]),
    ("Wp2", [K2, K2]), ("Up2", [K2, K2]),
    ("We1", [HID, HID]), ("Ue1", [HID, HID]),
    ("We2", [HID, HID]), ("Ue2", [HID, HID]),
    ("Wc1", [HID, HID]), ("Uc1", [HID, HID]),
    ("Wc2", [HID, OUT]), ("Uc2", [HID, OUT]),
]


# ---------------------------------------------------------------------------
# Walrus workaround: this toolchain's walrus encodes at most ONE sync wait
# per instruction (single EVENTS slot) and errors out instead of splitting.
# Split any multi-wait instruction by hoisting extra waits onto fresh
# single-wait NOPs right before it on the same engine.
# ---------------------------------------------------------------------------
_mw_ctr = [0]


def _legalize_multiwait(nc):
    for func in nc.m.functions:
        for bb in func.blocks:
            insts = bb.instructions
            new = []
            changed = False
            for ins in insts:
                si = getattr(ins, "sync_info", None)
                waits = list(si.on_wait) if (si and si.on_wait) else []
                if len(waits) > 1:
                    changed = True
                    for w in waits[:-1]:
                        _mw_ctr[0] += 1
                        nop = mybir.InstNoOp(
                            name=f"mwfix-{_mw_ctr[0]}",
                            engine=ins.engine,
                            ins=[],
                            outs=[],
                            sync_info=mybir.SyncInfo(on_wait=[w], on_update=[]),
                            bass_nofuse=True,
                        )
                        nc.register_instruction(nop, overwrite=True)
                        new.append(nop)
                    si.on_wait = [waits[-1]]
                new.append(ins)
            if changed:
                bb.instructions[:] = new


# ---------------------------------------------------------------------------
# Device program
# ---------------------------------------------------------------------------
def build_nc():
    nc = bass.Bass()

    # packed features: xt1 covers node cols 0..1151 (g1 + mixed chunk),
    # xt2 covers cols 1024..2047 (mixed chunk + g2); zeros elsewhere.
    xt1 = nc.dram_tensor("xt1", [BPC, IN_DIM, 9 * 128], BF16, kind="ExternalInput")
    xt2 = nc.dram_tensor("xt2", [BPC, IN_DIM, 8 * 128], BF16, kind="ExternalInput")
    adj1 = nc.dram_tensor("adj1", [BPC, 8, 128, W1COLS], FP8, kind="ExternalInput")
    adj2 = nc.dram_tensor("adj2", [BPC, 8, 128, MN], FP8, kind="ExternalInput")
    # all weights in two packed tensors (one DMA each: dispatch is ~600ns/DMA)
    wproj = nc.dram_tensor("wproj", [IN_DIM, 2 * PROJ_N], BF16,
                           kind="ExternalInput")
    l2w_cols = sum(shp[1] for _, shp in L2_W_SHAPES)
    wl2cat = nc.dram_tensor("wl2cat", [HID, l2w_cols], BF16,
                            kind="ExternalInput")
    out = nc.dram_tensor("out", [OUT, BPC], F32, kind="ExternalOutput")

    with tile.TileContext(nc) as tc:
        with (
            tc.tile_pool(name="const", bufs=1) as cpool,
            tc.tile_pool(name="xt", bufs=2) as xtpool,
            tc.tile_pool(name="slab", bufs=2) as slabpool,
            tc.tile_pool(name="sh", bufs=32) as shpool,
            tc.tile_pool(name="tb", bufs=32) as tbpool,
            tc.tile_pool(name="l2", bufs=2) as l2pool,
            tc.tile_pool(name="smx", bufs=4) as smx,
            tc.tile_pool(name="acc", bufs=4, space="PSUM") as accp,
            tc.tile_pool(name="ps", bufs=4, space="PSUM") as ps,
        ):
            # ---- constants (2 packed DMAs) ----
            wp_sb = cpool.tile([IN_DIM, 2 * PROJ_N], BF16, tag="wp")
            nc.sync.dma_start(out=wp_sb[:], in_=wproj[:])
            wp1_sb = wp_sb[:, :PROJ_N]
            wp2_sb = wp_sb[:, PROJ_N:]
            wcat_sb = cpool.tile([HID, l2w_cols], BF16, tag="wl2")
            nc.sync.dma_start(out=wcat_sb[:], in_=wl2cat[:])
            wsb = {}
            c0 = 0
            for name, shp in L2_W_SHAPES:
                wsb[name] = wcat_sb[:shp[0], c0:c0 + shp[1]]
                c0 += shp[1]
            warm = cpool.tile([128, 128], BF16, tag="warm")
            nc.gpsimd.memset(warm[:], 0.0)
            ones_col = cpool.tile([K2, 1], BF16, tag="ones_col")
            nc.gpsimd.memset(ones_col[:], 1.0)
            out_sb = cpool.tile([OUT, BPC], F32, tag="out_sb")

            # ---- input DMAs (2 xt + 2 merged adj per batch) ----
            xt_sb = []
            slabs = []   # [b][v] -> (tile, col offset of the v-slab)
            for b in range(BPC):
                x1t_sb = xtpool.tile([IN_DIM, 9 * 128], BF16, tag="xt1",
                                     name=f"xt1_{b}")
                x2t_sb = xtpool.tile([IN_DIM, 8 * 128], BF16, tag="xt2",
                                     name=f"xt2_{b}")
                nc.sync.dma_start(out=x1t_sb[:], in_=xt1[b])
                nc.sync.dma_start(out=x2t_sb[:], in_=xt2[b])
                xt_sb.append((x1t_sb, x2t_sb))
                sl1 = slabpool.tile([128, 8, W1COLS], FP8, tag="s1",
                                    name=f"slab1_{b}")
                nc.sync.dma_start(out=sl1[:],
                                  in_=adj1[b].rearrange("v p u -> p v u"))
                sl2 = slabpool.tile([128, 8, MN], FP8, tag="s2",
                                    name=f"slab2_{b}")
                nc.sync.dma_start(out=sl2[:],
                                  in_=adj2[b].rearrange("v p u -> p v u"))
                sl = [(sl1, v) for v in range(8)]
                sl += [(sl2, v) for v in range(8)]
                slabs.append(sl)

            # ---- PE warmup (HAM ramp during initial DMAs) ----
            for i in range(24):
                pw = ps.tile([128, 128], F32, tag="mm", name=f"warmps{i}")
                nc.tensor.matmul(pw[:], lhsT=warm[:], rhs=warm[:],
                                 start=True, stop=True)

            # ---- per-batch state ----
            S = [[None] * 16 for _ in range(BPC)]   # softmax score tiles (bf16)
            H = [[None] * 16 for _ in range(BPC)]   # relu embedding tiles (bf16)
            T = [[None] * 16 for _ in range(BPC)]   # t = adj@s tiles (bf16)
            A1T = [[None, None] for _ in range(BPC)]
            X1T = [None] * BPC

            def proj(b):
                """Projections + softmax/relu for all 16 node chunks."""
                x1t_sb, x2t_sb = xt_sb[b]
                for c in range(16):
                    p = ps.tile([128, PROJ_N], F32, tag="mm", name=f"pj{b}_{c}")
                    if c <= 7:
                        nc.tensor.matmul(p[:], lhsT=x1t_sb[:, c * 128:(c + 1) * 128],
                                         rhs=wp1_sb[:], start=True, stop=True)
                    elif c == 8:
                        nc.tensor.matmul(p[:], lhsT=x1t_sb[:, 1024:1152],
                                         rhs=wp1_sb[:], start=True, stop=False)
                        nc.tensor.matmul(p[:], lhsT=x2t_sb[:, 0:128],
                                         rhs=wp2_sb[:], start=False, stop=True)
                    else:
                        nc.tensor.matmul(p[:],
                                         lhsT=x2t_sb[:, (c - 8) * 128:(c - 7) * 128],
                                         rhs=wp2_sb[:], start=True, stop=True)
                    nmax = smx.tile([128, 1], F32, tag="nmax", name=f"nm{b}_{c}")
                    nc.vector.reduce_max(out=nmax[:], in_=p[:, :K1],
                                         axis=mybir.AxisListType.X, negate=True)
                    sbf = shpool.tile([128, K1], BF16, tag="s", name=f"s{b}_{c}")
                    ssum = smx.tile([128, 1], F32, tag="ssum", name=f"ss{b}_{c}")
                    nc.scalar.activation(out=sbf[:], in_=p[:, :K1],
                                         func=mybir.ActivationFunctionType.Exp,
                                         bias=nmax[:], scale=1.0,
                                         accum_out=ssum[:])
                    rinv = smx.tile([128, 1], F32, tag="rinv", name=f"ri{b}_{c}")
                    nc.vector.reciprocal(out=rinv[:], in_=ssum[:])
                    nc.vector.tensor_scalar_mul(out=sbf[:], in0=sbf[:],
                                                scalar1=rinv[:])
                    S[b][c] = sbf
                    hbf = shpool.tile([128, HID], BF16, tag="h", name=f"h{b}_{c}")
                    nc.scalar.activation(out=hbf[:], in_=p[:, HOFF:PROJ_N],
                                         func=mybir.ActivationFunctionType.Relu)
                    H[b][c] = hbf

            def t_pass(b, u_lo):
                """u-chunks u_lo..u_lo+3, one PSUM bank each (start=True is a
                bank-wide has_written clear, so groups never share a bank).
                u-chunks <=8 contract over all 16 v; >8 only v 8..15."""
                accs, v0s = [], []
                for i in range(4):
                    u = u_lo + i
                    a = accp.tile([128, K1], F32, tag="acc",
                                  name=f"acc{b}_{u}")
                    accs.append(a)
                    v0s.append(0 if u <= 8 else 8)
                for v in range(16):
                    sl, vi = slabs[b][v]
                    for i in range(4):
                        u = u_lo + i
                        if v < v0s[i]:
                            continue
                        usl = slice(u * 128, (u + 1) * 128)
                        nc.tensor.matmul(accs[i][:], lhsT=sl[:, vi, usl],
                                         rhs=S[b][v][:],
                                         start=(v == v0s[i]), stop=(v == 15))
                return accs

            def t_copies(b, accs, u_lo):
                """One pass's PSUM accumulators -> bf16 t tiles (vector engine).
                Emitted immediately after the pass so the acc slots free up for
                the next pass without a cross-engine ordering cycle."""
                for i in range(4):
                    u = u_lo + i
                    tbf = tbpool.tile([128, K1], BF16, tag="t", name=f"t{b}_{u}")
                    nc.vector.tensor_copy(out=tbf[:], in_=accs[i][:])
                    T[b][u] = tbf

            def epilogue(b):
                """a1t = t^T s  [205,205];  x1t = h^T s  [64,205] (bf16)."""
                for mi, (m0, msz) in enumerate(_M2):
                    pa = ps.tile([128, K1], F32, tag="mm", name=f"pa1t{b}_{mi}")
                    for v in range(16):
                        nc.tensor.matmul(pa[:msz, :],
                                         lhsT=T[b][v][:, m0:m0 + msz],
                                         rhs=S[b][v][:],
                                         start=(v == 0), stop=(v == 15))
                    asb = l2pool.tile([128, K1], BF16, tag=f"a1t{mi}",
                                      name=f"a1t{b}_{mi}")
                    nc.vector.tensor_copy(out=asb[:msz, :], in_=pa[:msz, :])
                    A1T[b][mi] = asb
                px = ps.tile([HID, K1], F32, tag="mm", name=f"px1t{b}")
                for v in range(16):
                    nc.tensor.matmul(px[:], lhsT=H[b][v][:], rhs=S[b][v][:],
                                     start=(v == 0), stop=(v == 15))
                xsb = l2pool.tile([HID, K1], BF16, tag="x1t", name=f"x1t{b}")
                nc.vector.tensor_copy(out=xsb[:], in_=px[:])
                X1T[b] = xsb

            # =============== level-2 (transpose-free), interleaved ===========
            # helper: matmul accumulate list of (lhsT, rhs) into one psum, then
            # copy (or relu) into an sbuf tile of given dtype.
            def mm_chain(b, srcs, osh, tag, relu=False, psname=None):
                p = ps.tile(list(osh), F32, tag="mm", name=f"p{tag}{b}")
                n = len(srcs)
                for i, (lt, rw) in enumerate(srcs):
                    nc.tensor.matmul(p[:], lhsT=lt, rhs=rw,
                                     start=(i == 0), stop=(i == n - 1))
                o = l2pool.tile(list(osh), BF16, tag=tag, name=f"{tag}{b}")
                if relu:
                    nc.scalar.activation(out=o[:], in_=p[:],
                                         func=mybir.ActivationFunctionType.Relu)
                else:
                    nc.vector.tensor_copy(out=o[:], in_=p[:])
                return o

            ctx = [dict() for _ in range(BPC)]

            def l2_z1(b, W1n, pfx):
                c = ctx[b]
                c[pfx + "z1"] = [
                    mm_chain(b, [(X1T[b][:, m0:m0 + msz], wsb[W1n][:])],
                             (msz, wsb[W1n].shape[1]), f"{pfx}z1_{mi}")
                    for mi, (m0, msz) in enumerate(_M2)]

            def l2_hht(b, U1n, pfx):
                c = ctx[b]
                z1 = c[pfx + "z1"]
                n1 = wsb[U1n].shape[1]
                srcs = [(z1[0][:], A1T[b][0][:]),
                        (z1[1][:77, :], A1T[b][1][:77, :]),
                        (wsb[U1n][:], X1T[b][:])]
                c[pfx + "hht"] = mm_chain(b, srcs, (n1, K1), f"{pfx}hht",
                                          relu=True)

            def l2_z2(b, W2n, pfx):
                c = ctx[b]
                hht = c[pfx + "hht"]
                c[pfx + "z2"] = [
                    mm_chain(b, [(hht[:, m0:m0 + msz], wsb[W2n][:])],
                             (msz, wsb[W2n].shape[1]), f"{pfx}z2_{mi}")
                    for mi, (m0, msz) in enumerate(_M2)]

            def l2_out(b, U2n, pfx, to_psum=False):
                """o = a1 @ z2 + hh @ U2, per m-tile. Returns psum tiles if
                to_psum (softmax consumes psum directly), else sbuf tiles."""
                c = ctx[b]
                z2 = c[pfx + "z2"]
                hht = c[pfx + "hht"]
                n2 = wsb[U2n].shape[1]
                outs = []
                for mi, (m0, msz) in enumerate(_M2):
                    msl = slice(m0, m0 + msz)
                    p = ps.tile([128, n2], F32, tag="mm", name=f"po{pfx}{b}{mi}")
                    nc.tensor.matmul(p[:msz, :], lhsT=A1T[b][0][:, msl],
                                     rhs=z2[0][:], start=True, stop=False)
                    nc.tensor.matmul(p[:msz, :], lhsT=A1T[b][1][:77, msl],
                                     rhs=z2[1][:77, :], start=False, stop=False)
                    nc.tensor.matmul(p[:msz, :], lhsT=hht[:, msl],
                                     rhs=wsb[U2n][:], start=False, stop=True)
                    if to_psum:
                        outs.append(p)
                    else:
                        o = l2pool.tile([128, n2], BF16, tag=f"{pfx}o_{mi}",
                                        name=f"{pfx}o{b}_{mi}")
                        nc.vector.tensor_copy(out=o[:msz, :], in_=p[:msz, :])
                        outs.append(o)
                c[pfx + "o"] = outs

            def l2_softmax(b):
                """softmax over K2 on the s2 psum tiles -> sm2 bf16 tiles."""
                c = ctx[b]
                sm = []
                for mi, (m0, msz) in enumerate(_M2):
                    p = c["o"][mi]
                    nmax = smx.tile([128, 1], F32, tag="nmax", name=f"l2nm{b}{mi}")
                    nc.vector.reduce_max(out=nmax[:msz], in_=p[:msz, :],
                                         axis=mybir.AxisListType.X, negate=True)
                    e = l2pool.tile([128, K2], BF16, tag=f"sm2_{mi}",
                                    name=f"sm2{b}_{mi}")
                    ssum = smx.tile([128, 1], F32, tag="ssum", name=f"l2ss{b}{mi}")
                    nc.scalar.activation(out=e[:msz, :], in_=p[:msz, :],
                                         func=mybir.ActivationFunctionType.Exp,
                                         bias=nmax[:msz], scale=1.0,
                                         accum_out=ssum[:msz])
                    rinv = smx.tile([128, 1], F32, tag="rinv", name=f"l2ri{b}{mi}")
                    nc.vector.reciprocal(out=rinv[:msz], in_=ssum[:msz])
                    nc.vector.tensor_scalar_mul(out=e[:msz, :], in0=e[:msz, :],
                                                scalar1=rinv[:msz])
                    sm.append(e)
                c["sm2"] = sm

            def l2_pool_stage(b):
                c = ctx[b]
                sm2 = c["sm2"]
                x1e = c["xo"]
                c["x2t"] = mm_chain(
                    b, [(x1e[0][:], sm2[0][:]), (x1e[1][:77, :], sm2[1][:77, :])],
                    (HID, K2), "x2t")
                y = []
                for mi, (m0, msz) in enumerate(_M2):
                    msl = slice(m0, m0 + msz)
                    y.append(mm_chain(
                        b, [(A1T[b][0][:, msl], sm2[0][:]),
                            (A1T[b][1][:77, msl], sm2[1][:77, :])],
                        (msz, K2), f"y_{mi}"))
                c["a2t"] = mm_chain(
                    b, [(y[0][:], sm2[0][:]), (y[1][:77, :], sm2[1][:77, :])],
                    (K2, K2), "a2t")

            def l2_final(b):
                c = ctx[b]
                x2t, a2t = c["x2t"], c["a2t"]
                z = mm_chain(b, [(x2t[:], wsb["Wc1"][:])], (K2, HID), "fz")
                h2t = mm_chain(b, [(z[:], a2t[:]), (wsb["Uc1"][:], x2t[:])],
                               (HID, K2), "fh2t", relu=True)
                z2f = mm_chain(b, [(h2t[:], wsb["Wc2"][:])], (K2, OUT), "fz2")
                onodes = mm_chain(b, [(a2t[:], z2f[:]), (h2t[:], wsb["Uc2"][:])],
                                  (K2, OUT), "fon")
                pm = ps.tile([OUT, 1], F32, tag="mm", name=f"pm{b}")
                nc.tensor.matmul(pm[:], lhsT=onodes[:], rhs=ones_col[:],
                                 start=True, stop=True)
                nc.scalar.activation(out=out_sb[:, b:b + 1], in_=pm[:],
                                     func=mybir.ActivationFunctionType.Copy,
                                     scale=1.0 / K2)

            # ---------------- emission schedule ----------------
            proj(0)
            for u_lo in (0, 4, 8, 12):
                a_ = t_pass(0, u_lo)
                t_copies(0, a_, u_lo)
            proj(1)
            epilogue(0)
            for u_lo in (0, 4, 8, 12):
                a_ = t_pass(1, u_lo)
                t_copies(1, a_, u_lo)
            epilogue(1)

            for b in range(BPC):
                l2_z1(b, "Wp1", "")
            for b in range(BPC):
                l2_hht(b, "Up1", "")
            for b in range(BPC):
                l2_z2(b, "Wp2", "")
            for b in range(BPC):
                l2_out(b, "Up2", "", to_psum=True)
            for b in range(BPC):
                l2_softmax(b)
            for b in range(BPC):
                l2_z1(b, "We1", "x")
            for b in range(BPC):
                l2_hht(b, "Ue1", "x")
            for b in range(BPC):
                l2_z2(b, "We2", "x")
            for b in range(BPC):
                l2_out(b, "Ue2", "x", to_psum=False)
            for b in range(BPC):
                l2_pool_stage(b)
            for b in range(BPC):
                l2_final(b)

            nc.sync.dma_start(out=out[:], in_=out_sb[:])

    _legalize_multiwait(nc)
    return nc


# ---------------------------------------------------------------------------
# Host side
# ---------------------------------------------------------------------------
def _prep_inputs(inputs):
    inp = {k: np.asarray(v) for k, v in inputs.items()}
    sl1 = inp["slice_g1"].astype(np.int64)
    sl2 = inp["slice_g2"].astype(np.int64)
    b1 = inp["batch_g1"].astype(np.int64)
    b2 = inp["batch_g2"].astype(np.int64)
    n1 = np.diff(sl1)
    pos1 = np.arange(inp["x_g1"].shape[0], dtype=np.int64) - sl1[b1]
    pos2 = (np.arange(inp["x_g2"].shape[0], dtype=np.int64) - sl2[b2]
            + n1[b2])

    # packed dense transposed features per batch (g1: cols 0..1151,
    # g2: original cols 1024..2047 stored at offset -1024), bf16
    xt1 = np.zeros((B, IN_DIM, 9 * 128), np.float32)
    xt2 = np.zeros((B, IN_DIM, 8 * 128), np.float32)
    xg1t = inp["x_g1"].T
    xg2t = inp["x_g2"].T
    for b in range(B):
        r1 = slice(sl1[b], sl1[b + 1])
        xt1[b][:, pos1[r1]] = xg1t[:, r1]
        r2 = slice(sl2[b], sl2[b + 1])
        xt2[b][:, pos2[r2] - 1024] = xg2t[:, r2]
    xt1 = xt1.astype(ml_dtypes.bfloat16)
    xt2 = xt2.astype(ml_dtypes.bfloat16)

    # transposed dense adjacency, fp8e4 (1.0 = 0x38), one per batch.
    # layout: [dst, src]; split into trimmed dst-chunks 0..7 / full 8..15
    e1, e2, eh = inp["edge_g1"], inp["edge_g2"], inp["edge_h"]
    eb = np.concatenate([b1[e1[0]], b2[e2[0]], b1[eh[0]]]).astype(np.int64)
    erow = np.concatenate([pos1[e1[0]], pos2[e2[0]], pos1[eh[0]]])
    ecol = np.concatenate([pos1[e1[1]], pos2[e2[1]], pos2[eh[1]]])
    adj_u8 = np.zeros((B, MN, MN), np.uint8)           # [b, dst, src]
    adj_u8[eb, ecol, erow] = 0x38
    adj3 = adj_u8.reshape(B, 16, 128, MN)
    adj1 = np.ascontiguousarray(adj3[:, :8, :, :W1COLS]).view(ml_dtypes.float8_e4m3)
    adj2 = np.ascontiguousarray(adj3[:, 8:, :, :]).view(ml_dtypes.float8_e4m3)

    # projection weights: [205 scores | 3 zero pad | 64 emb] x2, bf16, packed
    wproj = np.zeros((IN_DIM, 2 * PROJ_N), np.float32)
    wproj[:, :K1] = inp["W_pool_g1"]
    wproj[:, HOFF:PROJ_N] = inp["W_emb_g1"]
    wproj[:, PROJ_N:PROJ_N + K1] = inp["W_pool_g2"]
    wproj[:, PROJ_N + HOFF:] = inp["W_emb_g2"]
    l2w_cols = sum(shp[1] for _, shp in L2_W_SHAPES)
    wl2cat = np.zeros((HID, l2w_cols), np.float32)
    c0 = 0
    for name, shp in L2_W_SHAPES:
        wl2cat[:shp[0], c0:c0 + shp[1]] = inp[name]
        c0 += shp[1]
    shared = dict(
        wproj=wproj.astype(ml_dtypes.bfloat16),
        wl2cat=wl2cat.astype(ml_dtypes.bfloat16),
    )
    in_maps = []
    for c in range(NCORES):
        bs = slice(c * BPC, (c + 1) * BPC)
        in_maps.append(dict(
            xt1=np.ascontiguousarray(xt1[bs]),
            xt2=np.ascontiguousarray(xt2[bs]),
            adj1=np.ascontiguousarray(adj1[bs]),
            adj2=np.ascontiguousarray(adj2[bs]),
            **shared,
        ))
    return in_maps


_NC_CACHE = {}


def run(inputs, trace=False, tmpdir=None):
    if "nc" not in _NC_CACHE:
        _NC_CACHE["nc"] = build_nc()
    nc = _NC_CACHE["nc"]
    in_maps = _prep_inputs(inputs)
    res = run_bass_kernel_spmd(nc, in_maps, list(range(NCORES)),
                               trace=trace, tmpdir=tmpdir)
    y = np.zeros((B, OUT), np.float32)
    for c in range(NCORES):
        o = res.results[c]["out"]       # [OUT, BPC]
        for b in range(BPC):
            y[c * BPC + b] = o[:, b]
    return y, res


def kernel(**inputs):
    y, _ = run(inputs)
    return y


# revision 32
# speedup vs baseline: 1.0249x; 1.0249x over previous
"""DiffPool GNN forward on 8 Trainium2 NeuronCores.

Data-parallel over the batch dim (B=16 -> 2 batches per core). Each core
receives its two batches' dense node features (bf16, transposed) and dense
transposed adjacency (fp8e4, 0/1 exact), runs the DiffPool pipeline locally,
and emits its two [2] outputs.

Key structure vs the naive version:
  - adjacency ships as fp8 (half the HBM bytes) and is consumed directly as
    the stationary matmul operand against bf16 softmax scores (mixed-dtype
    matmul, numerically exact for 0/1 weights).
  - the t = adj @ s GEMM is v-outer (streams slabs as they arrive from HBM)
    with 16 PSUM accumulators dual-packed into 4 banks x 2 passes, and skips
    the structurally-zero blocks (g2 rows never see g1 columns): 200 matmuls
    instead of 256, slabs 0..7 shipped at 1152 instead of 2048 columns.
  - level-2 GNNs are algebraically restructured to be transpose-free (every
    operand is produced directly in the layout the next matmul needs).
  - the two batches' level-2 chains are interleaved to hide latency.
"""

import numpy as np
import ml_dtypes

import concourse.bass as bass
import concourse.mybir as mybir
from concourse import tile
from concourse.bass_utils import run_bass_kernel_spmd

# ---------------------------------------------------------------------------
# Problem constants (hardcoded per spec)
# ---------------------------------------------------------------------------
B = 16
NCORES = 8
BPC = B // NCORES          # batches per core
MN = 2048                  # MAX_NODES
IN_DIM = 128
HID = 64
OUT = 2
K1 = 205
K2 = 21
N1P = 1100                 # g1 nodes per batch
N2P = 900                  # g2 nodes per batch
W1COLS = 1152              # trimmed slab width for dst-chunks 0..7
PROJ_N = 272               # 205 scores | 3 pad | 64 emb  (relu offset 8B-aligned)
HOFF = 208                 # emb column offset inside proj psum

F32 = mybir.dt.float32
BF16 = mybir.dt.bfloat16
FP8 = mybir.dt.float8e4

_M2 = ((0, 128), (128, K1 - 128))      # row tiling of a 205-row matrix

L2_W_SHAPES = [
    ("Wp1", [HID, K2]), ("Up1", [HID, K2]),
    ("Wp2", [K2, K2]), ("Up2", [K2, K2]),
    ("We1", [HID, HID]), ("Ue1", [HID, HID]),
    ("We2", [HID, HID]), ("Ue2", [HID, HID]),
    ("Wc1", [HID, HID]), ("Uc1", [HID, HID]),
    ("Wc2", [HID, OUT]), ("Uc2", [HID, OUT]),
]


# ---------------------------------------------------------------------------
# Walrus workaround: this toolchain's walrus encodes at most ONE sync wait
# per instruction (single EVENTS slot) and errors out instead of splitting.
# Split any multi-wait instruction by hoisting extra waits onto fresh
# single-wait NOPs right before it on the same engine.
# ---------------------------------------------------------------------------
_mw_ctr = [0]


def _legalize_multiwait(nc):
    for func in nc.m.functions:
        for bb in func.blocks:
            insts = bb.instructions
            new = []
            changed = False
            for ins in insts:
                si = getattr(ins, "sync_info", None)
                waits = list(si.on_wait) if (si and si.on_wait) else []
                if len(waits) > 1:
                    changed = True
                    for w in waits[:-1]:
                        _mw_ctr[0] += 1
                        nop = mybir.InstNoOp(
                            name=f"mwfix-{_mw_ctr[0]}",
                            engine=ins.engine,
                            ins=[],
                            outs=[],
                            sync_info=mybir.SyncInfo(on_wait=[w], on_update=[]),
                            bass_nofuse=True,
                        )
                        nc.register_instruction(nop, overwrite=True)
                        new.append(nop)
                    si.on_wait = [waits[-1]]
                new.append(ins)
            if changed:
                bb.instructions[:] = new


# ---------------------------------------------------------------------------
# Device program
# ---------------------------------------------------------------------------
def build_nc():
    nc = bass.Bass()

    # packed features: xt1 covers node cols 0..1151 (g1 + mixed chunk),
    # xt2 covers cols 1024..2047 (mixed chunk + g2); zeros elsewhere.
    xt1 = nc.dram_tensor("xt1", [BPC, IN_DIM, 9 * 128], BF16, kind="ExternalInput")
    xt2 = nc.dram_tensor("xt2", [BPC, IN_DIM, 8 * 128], BF16, kind="ExternalInput")
    adj1 = nc.dram_tensor("adj1", [BPC, 8, 128, W1COLS], FP8, kind="ExternalInput")
    adj2 = nc.dram_tensor("adj2", [BPC, 8, 128, MN], FP8, kind="ExternalInput")
    # all weights in two packed tensors (one DMA each: dispatch is ~600ns/DMA)
    wproj = nc.dram_tensor("wproj", [IN_DIM, 2 * PROJ_N], BF16,
                           kind="ExternalInput")
    l2w_cols = sum(shp[1] for _, shp in L2_W_SHAPES)
    wl2cat = nc.dram_tensor("wl2cat", [HID, l2w_cols], BF16,
                            kind="ExternalInput")
    # f32 copies of the final-GNN weights (the late stages run f32: their
    # matmuls are tiny but their values are huge, so bf16 rounding there
    # dominates the error budget)
    wl2f32 = nc.dram_tensor("wl2f32", [HID, 2 * HID + 2 * OUT], F32,
                            kind="ExternalInput")
    out = nc.dram_tensor("out", [OUT, BPC], F32, kind="ExternalOutput")

    with tile.TileContext(nc) as tc:
        with (
            tc.tile_pool(name="const", bufs=1) as cpool,
            tc.tile_pool(name="xt", bufs=2) as xtpool,
            tc.tile_pool(name="slab", bufs=2) as slabpool,
            tc.tile_pool(name="sh", bufs=32) as shpool,
            tc.tile_pool(name="tb", bufs=32) as tbpool,
            tc.tile_pool(name="l2", bufs=2) as l2pool,
            tc.tile_pool(name="smx", bufs=4) as smx,
            tc.tile_pool(name="acc", bufs=4, space="PSUM") as accp,
            tc.tile_pool(name="ps", bufs=4, space="PSUM") as ps,
        ):
            # ---- constants (2 packed DMAs) ----
            wp_sb = cpool.tile([IN_DIM, 2 * PROJ_N], BF16, tag="wp")
            nc.sync.dma_start(out=wp_sb[:], in_=wproj[:])
            wp1_sb = wp_sb[:, :PROJ_N]
            wp2_sb = wp_sb[:, PROJ_N:]
            wcat_sb = cpool.tile([HID, l2w_cols], BF16, tag="wl2")
            nc.sync.dma_start(out=wcat_sb[:], in_=wl2cat[:])
            wsb = {}
            c0 = 0
            for name, shp in L2_W_SHAPES:
                wsb[name] = wcat_sb[:shp[0], c0:c0 + shp[1]]
                c0 += shp[1]
            wf_sb = cpool.tile([HID, 2 * HID + 2 * OUT], F32, tag="wf")
            nc.sync.dma_start(out=wf_sb[:], in_=wl2f32[:])
            wsb["Wc1f"] = wf_sb[:, :HID]
            wsb["Uc1f"] = wf_sb[:, HID:2 * HID]
            wsb["Wc2f"] = wf_sb[:, 2 * HID:2 * HID + OUT]
            wsb["Uc2f"] = wf_sb[:, 2 * HID + OUT:]
            warm = cpool.tile([128, 128], BF16, tag="warm")
            nc.gpsimd.memset(warm[:], 0.0)
            ones_col = cpool.tile([K2, 1], F32, tag="ones_col")
            nc.gpsimd.memset(ones_col[:], 1.0)
            out_sb = cpool.tile([OUT, BPC], F32, tag="out_sb")

            # ---- input DMAs (2 xt + 2 merged adj per batch) ----
            xt_sb = []
            slabs = []   # [b][v] -> (tile, col offset of the v-slab)
            for b in range(BPC):
                x1t_sb = xtpool.tile([IN_DIM, 9 * 128], BF16, tag="xt1",
                                     name=f"xt1_{b}")
                x2t_sb = xtpool.tile([IN_DIM, 8 * 128], BF16, tag="xt2",
                                     name=f"xt2_{b}")
                nc.sync.dma_start(out=x1t_sb[:], in_=xt1[b])
                nc.sync.dma_start(out=x2t_sb[:], in_=xt2[b])
                xt_sb.append((x1t_sb, x2t_sb))
                sl1 = slabpool.tile([128, 8, W1COLS], FP8, tag="s1",
                                    name=f"slab1_{b}")
                nc.sync.dma_start(out=sl1[:],
                                  in_=adj1[b].rearrange("v p u -> p v u"))
                sl2 = slabpool.tile([128, 8, MN], FP8, tag="s2",
                                    name=f"slab2_{b}")
                nc.sync.dma_start(out=sl2[:],
                                  in_=adj2[b].rearrange("v p u -> p v u"))
                sl = [(sl1, v) for v in range(8)]
                sl += [(sl2, v) for v in range(8)]
                slabs.append(sl)

            # ---- PE warmup (HAM ramp during initial DMAs) ----
            for i in range(24):
                pw = ps.tile([128, 128], F32, tag="mm", name=f"warmps{i}")
                nc.tensor.matmul(pw[:], lhsT=warm[:], rhs=warm[:],
                                 start=True, stop=True)

            # ---- per-batch state ----
            S = [[None] * 16 for _ in range(BPC)]   # softmax score tiles (bf16)
            H = [[None] * 16 for _ in range(BPC)]   # relu embedding tiles (bf16)
            T = [[None] * 16 for _ in range(BPC)]   # t = adj@s tiles (bf16)
            A1T = [[None, None] for _ in range(BPC)]
            X1T = [None] * BPC

            def proj(b):
                """Projections + softmax/relu for all 16 node chunks."""
                x1t_sb, x2t_sb = xt_sb[b]
                for c in range(16):
                    p = ps.tile([128, PROJ_N], F32, tag="mm", name=f"pj{b}_{c}")
                    if c <= 7:
                        nc.tensor.matmul(p[:], lhsT=x1t_sb[:, c * 128:(c + 1) * 128],
                                         rhs=wp1_sb[:], start=True, stop=True)
                    elif c == 8:
                        nc.tensor.matmul(p[:], lhsT=x1t_sb[:, 1024:1152],
                                         rhs=wp1_sb[:], start=True, stop=False)
                        nc.tensor.matmul(p[:], lhsT=x2t_sb[:, 0:128],
                                         rhs=wp2_sb[:], start=False, stop=True)
                    else:
                        nc.tensor.matmul(p[:],
                                         lhsT=x2t_sb[:, (c - 8) * 128:(c - 7) * 128],
                                         rhs=wp2_sb[:], start=True, stop=True)
                    # scores are tiny (|x| < ~0.3): exp never overflows, so
                    # skip the usual max-subtraction (softmax shift-invariant)
                    sbf = shpool.tile([128, K1], BF16, tag="s", name=f"s{b}_{c}")
                    ssum = smx.tile([128, 1], F32, tag="ssum", name=f"ss{b}_{c}")
                    nc.scalar.activation(out=sbf[:], in_=p[:, :K1],
                                         func=mybir.ActivationFunctionType.Exp,
                                         scale=1.0,
                                         accum_out=ssum[:])
                    rinv = smx.tile([128, 1], F32, tag="rinv", name=f"ri{b}_{c}")
                    nc.vector.reciprocal(out=rinv[:], in_=ssum[:])
                    nc.vector.tensor_scalar_mul(out=sbf[:], in0=sbf[:],
                                                scalar1=rinv[:])
                    S[b][c] = sbf
                    hbf = shpool.tile([128, HID], BF16, tag="h", name=f"h{b}_{c}")
                    nc.scalar.activation(out=hbf[:], in_=p[:, HOFF:PROJ_N],
                                         func=mybir.ActivationFunctionType.Relu)
                    H[b][c] = hbf

            def t_pass(b, u_lo):
                """u-chunks u_lo..u_lo+3, one PSUM bank each (start=True is a
                bank-wide has_written clear, so groups never share a bank).
                u-chunks <=8 contract over all 16 v; >8 only v 8..15."""
                accs, v0s = [], []
                for i in range(4):
                    u = u_lo + i
                    a = accp.tile([128, K1], F32, tag="acc",
                                  name=f"acc{b}_{u}")
                    accs.append(a)
                    v0s.append(0 if u <= 8 else 8)
                for v in range(16):
                    sl, vi = slabs[b][v]
                    for i in range(4):
                        u = u_lo + i
                        if v < v0s[i]:
                            continue
                        usl = slice(u * 128, (u + 1) * 128)
                        nc.tensor.matmul(accs[i][:], lhsT=sl[:, vi, usl],
                                         rhs=S[b][v][:],
                                         start=(v == v0s[i]), stop=(v == 15))
                return accs

            def t_copies(b, accs, u_lo):
                """One pass's PSUM accumulators -> bf16 t tiles (vector engine).
                Emitted immediately after the pass so the acc slots free up for
                the next pass without a cross-engine ordering cycle."""
                for i in range(4):
                    u = u_lo + i
                    tbf = tbpool.tile([128, K1], BF16, tag="t", name=f"t{b}_{u}")
                    nc.vector.tensor_copy(out=tbf[:], in_=accs[i][:])
                    T[b][u] = tbf

            def epilogue(b):
                """a1t = t^T s  [205,205];  x1t = h^T s  [64,205] (bf16)."""
                for mi, (m0, msz) in enumerate(_M2):
                    pa = ps.tile([128, K1], F32, tag="mm", name=f"pa1t{b}_{mi}")
                    for v in range(16):
                        nc.tensor.matmul(pa[:msz, :],
                                         lhsT=T[b][v][:, m0:m0 + msz],
                                         rhs=S[b][v][:],
                                         start=(v == 0), stop=(v == 15))
                    asb = l2pool.tile([128, K1], BF16, tag=f"a1t{mi}",
                                      name=f"a1t{b}_{mi}")
                    nc.vector.tensor_copy(out=asb[:msz, :], in_=pa[:msz, :])
                    A1T[b][mi] = asb
                px = ps.tile([HID, K1], F32, tag="mm", name=f"px1t{b}")
                for v in range(16):
                    nc.tensor.matmul(px[:], lhsT=H[b][v][:], rhs=S[b][v][:],
                                     start=(v == 0), stop=(v == 15))
                xsb = l2pool.tile([HID, K1], BF16, tag="x1t", name=f"x1t{b}")
                nc.vector.tensor_copy(out=xsb[:], in_=px[:])
                X1T[b] = xsb

            # =============== level-2 (transpose-free), interleaved ===========
            # helper: matmul accumulate list of (lhsT, rhs) into one psum, then
            # copy (or relu) into an sbuf tile of given dtype.
            def mm_chain(b, srcs, osh, tag, relu=False, dtype=BF16):
                p = ps.tile(list(osh), F32, tag="mm", name=f"p{tag}{b}")
                n = len(srcs)
                for i, (lt, rw) in enumerate(srcs):
                    nc.tensor.matmul(p[:], lhsT=lt, rhs=rw,
                                     start=(i == 0), stop=(i == n - 1))
                o = l2pool.tile(list(osh), dtype, tag=tag, name=f"{tag}{b}")
                if relu:
                    nc.scalar.activation(out=o[:], in_=p[:],
                                         func=mybir.ActivationFunctionType.Relu)
                else:
                    nc.vector.tensor_copy(out=o[:], in_=p[:])
                return o

            ctx = [dict() for _ in range(BPC)]

            def l2_z1(b, W1n, pfx):
                c = ctx[b]
                c[pfx + "z1"] = [
                    mm_chain(b, [(X1T[b][:, m0:m0 + msz], wsb[W1n][:])],
                             (msz, wsb[W1n].shape[1]), f"{pfx}z1_{mi}")
                    for mi, (m0, msz) in enumerate(_M2)]

            def l2_hht(b, U1n, pfx):
                c = ctx[b]
                z1 = c[pfx + "z1"]
                n1 = wsb[U1n].shape[1]
                srcs = [(z1[0][:], A1T[b][0][:]),
                        (z1[1][:77, :], A1T[b][1][:77, :]),
                        (wsb[U1n][:], X1T[b][:])]
                c[pfx + "hht"] = mm_chain(b, srcs, (n1, K1), f"{pfx}hht",
                                          relu=True)

            def l2_z2(b, W2n, pfx):
                c = ctx[b]
                hht = c[pfx + "hht"]
                c[pfx + "z2"] = [
                    mm_chain(b, [(hht[:, m0:m0 + msz], wsb[W2n][:])],
                             (msz, wsb[W2n].shape[1]), f"{pfx}z2_{mi}")
                    for mi, (m0, msz) in enumerate(_M2)]

            def l2_out(b, U2n, pfx, to_psum=False, dtype=BF16):
                """o = a1 @ z2 + hh @ U2, per m-tile. Returns psum tiles if
                to_psum (softmax consumes psum directly), else sbuf tiles."""
                c = ctx[b]
                z2 = c[pfx + "z2"]
                hht = c[pfx + "hht"]
                n2 = wsb[U2n].shape[1]
                outs = []
                for mi, (m0, msz) in enumerate(_M2):
                    msl = slice(m0, m0 + msz)
                    p = ps.tile([128, n2], F32, tag="mm", name=f"po{pfx}{b}{mi}")
                    nc.tensor.matmul(p[:msz, :], lhsT=A1T[b][0][:, msl],
                                     rhs=z2[0][:], start=True, stop=False)
                    nc.tensor.matmul(p[:msz, :], lhsT=A1T[b][1][:77, msl],
                                     rhs=z2[1][:77, :], start=False, stop=False)
                    nc.tensor.matmul(p[:msz, :], lhsT=hht[:, msl],
                                     rhs=wsb[U2n][:], start=False, stop=True)
                    if to_psum:
                        outs.append(p)
                    else:
                        o = l2pool.tile([128, n2], dtype, tag=f"{pfx}o_{mi}",
                                        name=f"{pfx}o{b}_{mi}")
                        nc.vector.tensor_copy(out=o[:msz, :], in_=p[:msz, :])
                        outs.append(o)
                c[pfx + "o"] = outs

            def l2_softmax(b):
                """softmax over K2 on the s2 psum tiles -> sm2 f32 + bf16."""
                c = ctx[b]
                sm, smb = [], []
                for mi, (m0, msz) in enumerate(_M2):
                    p = c["o"][mi]
                    nmax = smx.tile([128, 1], F32, tag="nmax", name=f"l2nm{b}{mi}")
                    nc.vector.reduce_max(out=nmax[:msz], in_=p[:msz, :],
                                         axis=mybir.AxisListType.X, negate=True)
                    e = l2pool.tile([128, K2], F32, tag=f"sm2_{mi}",
                                    name=f"sm2{b}_{mi}")
                    ssum = smx.tile([128, 1], F32, tag="ssum", name=f"l2ss{b}{mi}")
                    nc.scalar.activation(out=e[:msz, :], in_=p[:msz, :],
                                         func=mybir.ActivationFunctionType.Exp,
                                         bias=nmax[:msz], scale=1.0,
                                         accum_out=ssum[:msz])
                    rinv = smx.tile([128, 1], F32, tag="rinv", name=f"l2ri{b}{mi}")
                    nc.vector.reciprocal(out=rinv[:msz], in_=ssum[:msz])
                    nc.vector.tensor_scalar_mul(out=e[:msz, :], in0=e[:msz, :],
                                                scalar1=rinv[:msz])
                    eb = l2pool.tile([128, K2], BF16, tag=f"sm2b_{mi}",
                                     name=f"sm2b{b}_{mi}")
                    nc.vector.tensor_copy(out=eb[:msz, :], in_=e[:msz, :])
                    sm.append(e)
                    smb.append(eb)
                c["sm2"] = sm
                c["sm2b"] = smb

            def l2_pool_stage(b):
                c = ctx[b]
                sm2, sm2b = c["sm2"], c["sm2b"]
                x1e = c["xo"]
                c["x2t"] = mm_chain(
                    b, [(x1e[0][:], sm2[0][:]), (x1e[1][:77, :], sm2[1][:77, :])],
                    (HID, K2), "x2t", dtype=F32)
                y = []
                for mi, (m0, msz) in enumerate(_M2):
                    msl = slice(m0, m0 + msz)
                    y.append(mm_chain(
                        b, [(A1T[b][0][:, msl], sm2b[0][:]),
                            (A1T[b][1][:77, msl], sm2b[1][:77, :])],
                        (msz, K2), f"y_{mi}", dtype=F32))
                c["a2t"] = mm_chain(
                    b, [(y[0][:], sm2[0][:]), (y[1][:77, :], sm2[1][:77, :])],
                    (K2, K2), "a2t", dtype=F32)

            def l2_final(b):
                c = ctx[b]
                x2t, a2t = c["x2t"], c["a2t"]
                z = mm_chain(b, [(x2t[:], wsb["Wc1f"][:])], (K2, HID), "fz",
                             dtype=F32)
                h2t = mm_chain(b, [(z[:], a2t[:]), (wsb["Uc1f"][:], x2t[:])],
                               (HID, K2), "fh2t", relu=True, dtype=F32)
                z2f = mm_chain(b, [(h2t[:], wsb["Wc2f"][:])], (K2, OUT), "fz2",
                               dtype=F32)
                onodes = mm_chain(b, [(a2t[:], z2f[:]), (h2t[:], wsb["Uc2f"][:])],
                                  (K2, OUT), "fon", dtype=F32)
                pm = ps.tile([OUT, 1], F32, tag="mm", name=f"pm{b}")
                nc.tensor.matmul(pm[:], lhsT=onodes[:], rhs=ones_col[:],
                                 start=True, stop=True)
                nc.scalar.activation(out=out_sb[:, b:b + 1], in_=pm[:],
                                     func=mybir.ActivationFunctionType.Copy,
                                     scale=1.0 / K2)

            # ---------------- emission schedule ----------------
            proj(0)
            for u_lo in (0, 4, 8, 12):
                a_ = t_pass(0, u_lo)
                t_copies(0, a_, u_lo)
            proj(1)
            epilogue(0)
            # t1 passes with batch-0's level-2 chain woven between (each pass
            # is several us of PE work, plenty to hide the copy chains)
            a_ = t_pass(1, 0)
            t_copies(1, a_, 0)
            l2_z1(0, "Wp1", "")
            a_ = t_pass(1, 4)
            t_copies(1, a_, 4)
            l2_hht(0, "Up1", "")
            l2_z2(0, "Wp2", "")
            a_ = t_pass(1, 8)
            t_copies(1, a_, 8)
            l2_out(0, "Up2", "", to_psum=True)
            l2_softmax(0)
            a_ = t_pass(1, 12)
            t_copies(1, a_, 12)
            l2_z1(0, "We1", "x")
            l2_hht(0, "Ue1", "x")
            epilogue(1)
            l2_z2(0, "We2", "x")
            l2_out(0, "Ue2", "x", dtype=F32)
            l2_z1(1, "Wp1", "")
            l2_pool_stage(0)
            l2_hht(1, "Up1", "")
            l2_final(0)
            l2_z2(1, "Wp2", "")
            l2_out(1, "Up2", "", to_psum=True)
            l2_softmax(1)
            l2_z1(1, "We1", "x")
            l2_hht(1, "Ue1", "x")
            l2_z2(1, "We2", "x")
            l2_out(1, "Ue2", "x", dtype=F32)
            l2_pool_stage(1)
            l2_final(1)

            nc.sync.dma_start(out=out[:], in_=out_sb[:])

    _legalize_multiwait(nc)
    return nc


# ---------------------------------------------------------------------------
# Host side
# ---------------------------------------------------------------------------
def _prep_inputs(inputs):
    inp = {k: np.asarray(v) for k, v in inputs.items()}
    sl1 = inp["slice_g1"].astype(np.int64)
    sl2 = inp["slice_g2"].astype(np.int64)
    b1 = inp["batch_g1"].astype(np.int64)
    b2 = inp["batch_g2"].astype(np.int64)
    n1 = np.diff(sl1)
    pos1 = np.arange(inp["x_g1"].shape[0], dtype=np.int64) - sl1[b1]
    pos2 = (np.arange(inp["x_g2"].shape[0], dtype=np.int64) - sl2[b2]
            + n1[b2])

    # packed dense transposed features per batch (g1: cols 0..1151,
    # g2: original cols 1024..2047 stored at offset -1024), bf16
    xt1 = np.zeros((B, IN_DIM, 9 * 128), np.float32)
    xt2 = np.zeros((B, IN_DIM, 8 * 128), np.float32)
    xg1t = inp["x_g1"].T
    xg2t = inp["x_g2"].T
    for b in range(B):
        r1 = slice(sl1[b], sl1[b + 1])
        xt1[b][:, pos1[r1]] = xg1t[:, r1]
        r2 = slice(sl2[b], sl2[b + 1])
        xt2[b][:, pos2[r2] - 1024] = xg2t[:, r2]
    xt1 = xt1.astype(ml_dtypes.bfloat16)
    xt2 = xt2.astype(ml_dtypes.bfloat16)

    # transposed dense adjacency, fp8e4 (1.0 = 0x38), one per batch.
    # layout: [dst, src]; split into trimmed dst-chunks 0..7 / full 8..15
    e1, e2, eh = inp["edge_g1"], inp["edge_g2"], inp["edge_h"]
    eb = np.concatenate([b1[e1[0]], b2[e2[0]], b1[eh[0]]]).astype(np.int64)
    erow = np.concatenate([pos1[e1[0]], pos2[e2[0]], pos1[eh[0]]])
    ecol = np.concatenate([pos1[e1[1]], pos2[e2[1]], pos2[eh[1]]])
    adj_u8 = np.zeros((B, MN, MN), np.uint8)           # [b, dst, src]
    adj_u8[eb, ecol, erow] = 0x38
    adj3 = adj_u8.reshape(B, 16, 128, MN)
    adj1 = np.ascontiguousarray(adj3[:, :8, :, :W1COLS]).view(ml_dtypes.float8_e4m3)
    adj2 = np.ascontiguousarray(adj3[:, 8:, :, :]).view(ml_dtypes.float8_e4m3)

    # projection weights: [205 scores | 3 zero pad | 64 emb] x2, bf16, packed
    wproj = np.zeros((IN_DIM, 2 * PROJ_N), np.float32)
    wproj[:, :K1] = inp["W_pool_g1"]
    wproj[:, HOFF:PROJ_N] = inp["W_emb_g1"]
    wproj[:, PROJ_N:PROJ_N + K1] = inp["W_pool_g2"]
    wproj[:, PROJ_N + HOFF:] = inp["W_emb_g2"]
    l2w_cols = sum(shp[1] for _, shp in L2_W_SHAPES)
    wl2cat = np.zeros((HID, l2w_cols), np.float32)
    c0 = 0
    for name, shp in L2_W_SHAPES:
        wl2cat[:shp[0], c0:c0 + shp[1]] = inp[name]
        c0 += shp[1]
    wl2f32 = np.concatenate(
        [inp["Wc1"], inp["Uc1"], inp["Wc2"], inp["Uc2"]], axis=1
    ).astype(np.float32)
    shared = dict(
        wproj=wproj.astype(ml_dtypes.bfloat16),
        wl2cat=wl2cat.astype(ml_dtypes.bfloat16),
        wl2f32=wl2f32,
    )
    in_maps = []
    for c in range(NCORES):
        bs = slice(c * BPC, (c + 1) * BPC)
        in_maps.append(dict(
            xt1=np.ascontiguousarray(xt1[bs]),
            xt2=np.ascontiguousarray(xt2[bs]),
            adj1=np.ascontiguousarray(adj1[bs]),
            adj2=np.ascontiguousarray(adj2[bs]),
            **shared,
        ))
    return in_maps


_NC_CACHE = {}


def run(inputs, trace=False, tmpdir=None):
    if "nc" not in _NC_CACHE:
        _NC_CACHE["nc"] = build_nc()
    nc = _NC_CACHE["nc"]
    in_maps = _prep_inputs(inputs)
    res = run_bass_kernel_spmd(nc, in_maps, list(range(NCORES)),
                               trace=trace, tmpdir=tmpdir)
    y = np.zeros((B, OUT), np.float32)
    for c in range(NCORES):
        o = res.results[c]["out"]       # [OUT, BPC]
        for b in range(BPC):
            y[c * BPC + b] = o[:, b]
    return y, res


def kernel(**inputs):
    y, _ = run(inputs)
    return y
